# revision 5
# baseline (speedup 1.0000x reference)
"""AttentionMIL forward on 8 Trainium2 NeuronCores.

Data-parallel over the 16 bags (2 bags/core). Per bag:
  h1 = relu(LN(x @ W1 + b1))          x:[4096,1024] W1:[1024,512]
  h2 = relu(LN(h1 @ W2 + b2))
  s  = tanh(h2 @ Wa1 + ba1) @ wa2     (+ba2 dropped: softmax shift-invariant)
  attn = softmax(s); pooled = sum(attn * h2)
  logits = relu(pooled @ Wc1 + bc1) @ Wc2 + bc2

Matmuls run in float32r (reduced-precision fp32, ~1e-4 rel err, 4x faster
than fp32 on the PE). Tokens live on the partition axis so LN/softmax
reductions are free-axis ops; x is pre-transposed on the host so the only
on-device transposes are h1/h2 (PE transpose-mode).
"""

import numpy as np

B, N, D, H, C = 16, 4096, 1024, 512, 2
NCORES = 8
NB = B // NCORES       # bags per core
P = 128
NT = N // P            # token tiles per bag
DK = D // P            # k-chunks for D
HK = H // P            # k-chunks for H

_BUILD_CACHE = {}


def _build(flags):
    import concourse.bass as bass
    import concourse.mybir as mybir
    import concourse.tile as tile
    from concourse import bacc

    z_b1, aff1, z_b2, aff2, z_ba1 = flags
    f32 = mybir.dt.float32
    f32r = mybir.dt.float32r
    EPS = 1e-5

    nc = bacc.Bacc(None, target_bir_lowering=False, debug=False)

    xt = nc.dram_tensor("xt", [NB, D, N], f32, kind="ExternalInput")
    W1 = nc.dram_tensor("W1", [D, H], f32, kind="ExternalInput")
    W2 = nc.dram_tensor("W2", [H, H], f32, kind="ExternalInput")
    Wa1 = nc.dram_tensor("Wa1", [H, H], f32, kind="ExternalInput")
    wa2 = nc.dram_tensor("wa2", [H, 1], f32, kind="ExternalInput")
    Wc1 = nc.dram_tensor("Wc1", [H, H], f32, kind="ExternalInput")
    Wc2 = nc.dram_tensor("Wc2", [H, C], f32, kind="ExternalInput")
    bc1 = nc.dram_tensor("bc1", [H], f32, kind="ExternalInput")
    bc2 = nc.dram_tensor("bc2", [C], f32, kind="ExternalInput")
    b1 = g1 = be1 = b2 = g2 = be2 = ba1 = None
    if not z_b1:
        b1 = nc.dram_tensor("b1", [H], f32, kind="ExternalInput")
    if aff1:
        g1 = nc.dram_tensor("g1", [H], f32, kind="ExternalInput")
        be1 = nc.dram_tensor("beta1", [H], f32, kind="ExternalInput")
    if not z_b2:
        b2 = nc.dram_tensor("b2", [H], f32, kind="ExternalInput")
    if aff2:
        g2 = nc.dram_tensor("g2", [H], f32, kind="ExternalInput")
        be2 = nc.dram_tensor("beta2", [H], f32, kind="ExternalInput")
    if not z_ba1:
        ba1 = nc.dram_tensor("ba1", [H], f32, kind="ExternalInput")
    y = nc.dram_tensor("y", [NB, C], f32, kind="ExternalOutput")

    AX = mybir.AxisListType
    OP = mybir.AluOpType
    AF = mybir.ActivationFunctionType
    import concourse.bass_isa as bass_isa

    with tile.TileContext(nc) as tc:
        import contextlib
        ctx = contextlib.ExitStack()
        with ctx:
            wpool = ctx.enter_context(tc.tile_pool(name="wpool", bufs=1))
            stage = ctx.enter_context(tc.tile_pool(name="stage", bufs=1))
            xtp = ctx.enter_context(tc.tile_pool(name="xtp", bufs=3))
            xtr = ctx.enter_context(tc.tile_pool(name="xtr", bufs=2))
            h1p = ctx.enter_context(tc.tile_pool(name="h1p", bufs=2))
            htp = ctx.enter_context(tc.tile_pool(name="htp", bufs=3))
            ap_ = ctx.enter_context(tc.tile_pool(name="ap_", bufs=2))
            h2p = ctx.enter_context(tc.tile_pool(name="h2p", bufs=1))
            stats = ctx.enter_context(tc.tile_pool(name="stats", bufs=6))
            smallp = ctx.enter_context(tc.tile_pool(name="smallp", bufs=2))
            psmm = ctx.enter_context(tc.tile_pool(name="psmm", bufs=4, space="PSUM"))
            pstr = ctx.enter_context(tc.tile_pool(name="pstr", bufs=3, space="PSUM"))

            # ---- one-time init: identities, eps, weights in f32r ----
            ident_f = wpool.tile([P, P], f32)
            from concourse.masks import make_identity
            make_identity(nc, ident_f)
            ident_r = wpool.tile([P, P], f32r)
            nc.vector.tensor_copy(ident_r[:], ident_f[:])
            eps_t = wpool.tile([P, 1], f32)
            nc.vector.memset(eps_t, EPS)

            def load_conv(dram_ap, shape, name):
                st = stage.tile(list(shape), f32, tag="stage", name=f"st_{name}")
                nc.sync.dma_start(st[:], dram_ap)
                cv = wpool.tile(list(shape), f32r, name=f"wr_{name}")
                nc.vector.tensor_copy(cv[:], st[:])
                return cv

            w1r = load_conv(W1.rearrange("(k p) h -> p k h", p=P), (P, DK, H), "w1")
            w2r = load_conv(W2.rearrange("(k p) h -> p k h", p=P), (P, HK, H), "w2")
            war = load_conv(Wa1.rearrange("(k p) h -> p k h", p=P), (P, HK, H), "wa1")
            wc1r = load_conv(
                Wc1.rearrange("(k p) (m j) -> p k m j", p=P, j=P), (P, HK, HK, P), "wc1"
            )
            wc2r = load_conv(Wc2.rearrange("(k p) c -> p k c", p=P), (P, HK, C), "wc2")

            wa2_rep = wpool.tile([P, H], f32)
            nc.gpsimd.dma_start(
                wa2_rep[:], wa2.rearrange("h 1 -> 1 h").to_broadcast((P, H))
            )
            bc1t = wpool.tile([P, HK], f32)
            nc.sync.dma_start(bc1t[:], bc1.rearrange("(m p) -> p m", p=P))
            bc2t = wpool.tile([C, 1], f32)
            nc.sync.dma_start(bc2t[:], bc2[:, None])

            def rep(v, name):
                if v is None:
                    return None
                t = wpool.tile([P, H], f32, name=f"rep_{name}")
                nc.gpsimd.dma_start(t[:], v[None, :].to_broadcast((P, H)))
                return t

            b1_rep = rep(b1, "b1")
            g1_rep = rep(g1, "g1")
            be1_rep = rep(be1, "be1")
            b2_rep = rep(b2, "b2")
            g2_rep = rep(g2, "g2")
            be2_rep = rep(be2, "be2")
            ba1_rep = rep(ba1, "ba1")

            xt_part = xt.rearrange("b (k p) n -> b p k n", p=P)

            # -------- layernorm helper: stats from src, relu-apply into out --
            def ln_relu(src_ps, out_sb, b_rep, g_rep, be_rep, tag):
                # optionally add per-feature bias first (general path)
                if b_rep is not None:
                    t = ap_.tile([P, H], f32, tag=f"lnb_{tag}", name=f"lnb_{tag}")
                    nc.vector.tensor_add(t[:], src_ps[:], b_rep[:])
                    src = t
                else:
                    src = src_ps
                bn = stats.tile([P, 6], f32, tag="bn", name="bn")
                nc.vector.bn_stats(bn[:], src[:])
                mv = stats.tile([P, 2], f32, tag="mv", name="mv")
                nc.vector.bn_aggr(mv[:], bn[:])
                sd = stats.tile([P, 1], f32, tag="sd", name="sd")
                nc.scalar.activation(sd[:], mv[:, 1:2], AF.Sqrt, bias=eps_t[:])
                rstd = stats.tile([P, 1], f32, tag="rstd", name="rstd")
                nc.vector.reciprocal(rstd[:], sd[:])
                nmr = stats.tile([P, 1], f32, tag="nmr", name="nmr")
                nc.vector.tensor_scalar(
                    nmr[:], mv[:, 0:1], rstd[:], -1.0, op0=OP.mult, op1=OP.mult
                )
                if g_rep is None:
                    nc.scalar.activation(
                        out_sb, src[:], AF.Relu, bias=nmr[:], scale=rstd[:]
                    )
                else:
                    z = ap_.tile([P, H], f32, tag=f"lnz_{tag}", name=f"lnz_{tag}")
                    nc.vector.tensor_scalar(
                        z[:], src[:], mv[:, 0:1], rstd[:], op0=OP.subtract, op1=OP.mult
                    )
                    nc.vector.tensor_mul(z[:], z[:], g_rep[:])
                    nc.vector.tensor_add(z[:], z[:], be_rep[:])
                    nc.scalar.activation(out_sb, z[:], AF.Relu)

            logits_ps = None
            poolT_sb = smallp.tile([P, HK, NB], f32r, bufs=1)

            for b in range(NB):
                h2_res = h2p.tile([P, NT, H], f32r, tag="h2res", name="h2res")
                s_sc = smallp.tile([P, NT], f32, tag="s", name="s_sc")

                for i in range(NT):
                    xt_sb = xtp.tile([P, DK, P], f32, tag="xt", name="xt_sb")
                    nc.sync.dma_start(
                        xt_sb[:], xt_part[b, :, :, i * P : (i + 1) * P]
                    )
                    xt_r = xtr.tile([P, DK, P], f32r, tag="xtr", name="xt_r")
                    nc.scalar.copy(xt_r[:], xt_sb[:])

                    ps1 = psmm.tile([P, H], f32, tag="mm", name="ps1")
                    for k in range(DK):
                        nc.tensor.matmul(
                            ps1[:], xt_r[:, k, :], w1r[:, k, :],
                            start=(k == 0), stop=(k == DK - 1),
                        )
                    h1 = h1p.tile([P, H], f32r, tag="h1", name="h1")
                    ln_relu(ps1, h1[:], b1_rep, g1_rep, be1_rep, "1")

                    trp1 = pstr.tile([P, H], f32r, tag="tr", name="trp1")
                    for c in range(HK):
                        nc.tensor.transpose(
                            trp1[:, c * P : (c + 1) * P],
                            h1[:, c * P : (c + 1) * P],
                            ident_r[:],
                        )
                    h1T = htp.tile([P, HK, P], f32r, tag="h1T", name="h1T")
                    nc.vector.tensor_copy(h1T[:], trp1[:])

                    ps2 = psmm.tile([P, H], f32, tag="mm", name="ps2")
                    for k in range(HK):
                        nc.tensor.matmul(
                            ps2[:], h1T[:, k, :], w2r[:, k, :],
                            start=(k == 0), stop=(k == HK - 1),
                        )
                    ln_relu(ps2, h2_res[:, i, :], b2_rep, g2_rep, be2_rep, "2")

                    trp2 = pstr.tile([P, H], f32r, tag="tr", name="trp2")
                    for c in range(HK):
                        nc.tensor.transpose(
                            trp2[:, c * P : (c + 1) * P],
                            h2_res[:, i, c * P : (c + 1) * P],
                            ident_r[:],
                        )
                    h2T = htp.tile([P, HK, P], f32r, tag="h2T", name="h2T")
                    nc.vector.tensor_copy(h2T[:], trp2[:])

                    psa = psmm.tile([P, H], f32, tag="mm", name="psa")
                    for k in range(HK):
                        nc.tensor.matmul(
                            psa[:], h2T[:, k, :], war[:, k, :],
                            start=(k == 0), stop=(k == HK - 1),
                        )
                    a_t = ap_.tile([P, H], f32, tag="a", name="a_t")
                    if ba1_rep is not None:
                        nc.vector.tensor_add(a_t[:], psa[:], ba1_rep[:])
                        nc.scalar.activation(a_t[:], a_t[:], AF.Tanh)
                    else:
                        nc.scalar.activation(a_t[:], psa[:], AF.Tanh)
                    # s[:, i] = sum_h a * wa2
                    nc.vector.tensor_mul(a_t[:], a_t[:], wa2_rep[:])
                    nc.vector.tensor_reduce(
                        s_sc[:, i : i + 1], a_t[:], axis=AX.X, op=OP.add
                    )

                # ---- softmax over the bag + attention pooling ----
                rmax = stats.tile([P, 1], f32, tag="sd", name="rmax")
                nc.vector.tensor_reduce(rmax[:], s_sc[:], axis=AX.X, op=OP.max)
                gmax = stats.tile([P, 1], f32, tag="rstd", name="gmax")
                nc.gpsimd.partition_all_reduce(
                    gmax[:], rmax[:], channels=P, reduce_op=bass_isa.ReduceOp.max
                )
                ngmax = stats.tile([P, 1], f32, tag="nmr", name="ngmax")
                nc.vector.tensor_scalar_mul(ngmax[:], gmax[:], -1.0)
                p_t = smallp.tile([P, NT], f32r, tag="p", name="p_t")
                zrow = stats.tile([P, 1], f32, tag="sd", name="zrow")
                nc.scalar.activation(
                    p_t[:], s_sc[:], AF.Exp, bias=ngmax[:], scale=1.0,
                    accum_out=zrow[:],
                )
                zsum = stats.tile([P, 1], f32, tag="rstd", name="zsum")
                nc.gpsimd.partition_all_reduce(
                    zsum[:], zrow[:], channels=P, reduce_op=bass_isa.ReduceOp.add
                )
                rz = stats.tile([P, 1], f32, tag="nmr", name="rz")
                nc.vector.reciprocal(rz[:], zsum[:])
                # attn = p / Z  (f32r in-place)
                nc.vector.tensor_scalar_mul(p_t[:], p_t[:], rz[:])

                pool_ps = psmm.tile([1, H], f32, tag="mm", name="pool_ps")
                for i in range(NT):
                    nc.tensor.matmul(
                        pool_ps[:], p_t[:, i : i + 1], h2_res[:, i, :],
                        start=(i == 0), stop=(i == NT - 1),
                    )
                pooled_sb = smallp.tile([P, H], f32, tag="pooled", name="pooled_sb")
                nc.vector.memset(pooled_sb[:], 0.0)
                nc.vector.tensor_copy(pooled_sb[0:1, :], pool_ps[:])
                poolT_ps = pstr.tile([P, H], f32, tag="tr", name="poolT_ps")
                for c in range(HK):
                    nc.tensor.transpose(
                        poolT_ps[:, c * P : (c + 1) * P],
                        pooled_sb[:, c * P : (c + 1) * P],
                        ident_f[:],
                    )
                # gather cols {0, P, 2P, 3P} -> [P, HK]
                nc.vector.tensor_copy(
                    poolT_sb[:, :, b],
                    poolT_ps.rearrange("p (c j) -> p c j", j=P)[:, :, 0],
                )

            # ---- classifier over both bags ----
            rc_ps = psmm.tile([P, HK, NB], f32, tag="mm", name="rc_ps")
            for m in range(HK):
                for k in range(HK):
                    nc.tensor.matmul(
                        rc_ps[:, m, :], wc1r[:, k, m, :], poolT_sb[:, k, :],
                        start=(k == 0), stop=(k == HK - 1),
                    )
            rc_sb = smallp.tile([P, HK, NB], f32r, tag="rc", name="rc_sb")
            for m in range(HK):
                nc.scalar.activation(
                    rc_sb[:, m, :], rc_ps[:, m, :], AF.Relu,
                    bias=bc1t[:, m : m + 1], scale=1.0,
                )
            lg_ps = psmm.tile([C, NB], f32, tag="mm", name="lg_ps")
            for k in range(HK):
                nc.tensor.matmul(
                    lg_ps[:], wc2r[:, k, :], rc_sb[:, k, :],
                    start=(k == 0), stop=(k == HK - 1),
                )
            lg_sb = smallp.tile([C, NB], f32, tag="lg", name="lg_sb")
            nc.scalar.activation(
                lg_sb[:], lg_ps[:], AF.Identity, bias=bc2t[:], scale=1.0
            )
            with nc.allow_non_contiguous_dma(reason="4-element logits store"):
                nc.sync.dma_start(y.rearrange("b c -> c b"), lg_sb[:])

    nc.compile()
    return nc


def _get_program(flags):
    if flags not in _BUILD_CACHE:
        _BUILD_CACHE[flags] = _build(flags)
    return _BUILD_CACHE[flags]


def kernel(**inputs):
    import sys
    for pth in ("/opt/trn_rl_repo",):
        if pth not in sys.path:
            sys.path.append(pth)
    from concourse.bass_utils import run_bass_kernel_spmd

    x = np.asarray(inputs["x"], dtype=np.float32)
    names = ["W1", "b1", "g1", "beta1", "W2", "b2", "g2", "beta2",
             "Wa1", "ba1", "wa2", "ba2", "Wc1", "bc1", "Wc2", "bc2"]
    w = {k: np.asarray(inputs[k], dtype=np.float32) for k in names}

    z_b1 = bool((w["b1"] == 0).all())
    aff1 = not bool((w["g1"] == 1).all() and (w["beta1"] == 0).all())
    z_b2 = bool((w["b2"] == 0).all())
    aff2 = not bool((w["g2"] == 1).all() and (w["beta2"] == 0).all())
    z_ba1 = bool((w["ba1"] == 0).all())
    flags = (z_b1, aff1, z_b2, aff2, z_ba1)

    nc = _get_program(flags)

    in_maps = []
    for core in range(NCORES):
        shard = x[core * NB : (core + 1) * NB]          # [NB, N, D]
        xt = np.ascontiguousarray(shard.transpose(0, 2, 1))  # [NB, D, N]
        m = {
            "xt": xt,
            "W1": w["W1"], "W2": w["W2"], "Wa1": w["Wa1"],
            "wa2": w["wa2"].reshape(H, 1),
            "Wc1": w["Wc1"], "Wc2": w["Wc2"],
            "bc1": w["bc1"], "bc2": w["bc2"],
        }
        if not z_b1:
            m["b1"] = w["b1"]
        if aff1:
            m["g1"] = w["g1"]
            m["beta1"] = w["beta1"]
        if not z_b2:
            m["b2"] = w["b2"]
        if aff2:
            m["g2"] = w["g2"]
            m["beta2"] = w["beta2"]
        if not z_ba1:
            m["ba1"] = w["ba1"]
        in_maps.append(m)

    res = run_bass_kernel_spmd(nc, in_maps, core_ids=list(range(NCORES)))
    out = np.concatenate([res.results[i]["y"] for i in range(NCORES)], axis=0)
    return out.astype(np.float32)


# revision 35
# speedup vs baseline: 2.8104x; 2.8104x over previous
"""AttentionMIL forward on 8 Trainium2 NeuronCores.

Data-parallel over the 16 bags (2 bags/core). Per bag:
  h1 = relu(LN(x @ W1 + b1))          x:[4096,1024] W1:[1024,512]
  h2 = relu(LN(h1 @ W2 + b2))
  s  = tanh(h2 @ Wa1 + ba1) @ wa2     (+ba2 dropped: softmax shift-invariant)
  attn = softmax(s); pooled = sum(attn * h2)
  logits = relu(pooled @ Wc1 + bc1) @ Wc2 + bc2

Matmuls run in float32r (reduced-precision fp32, ~4e-4 rel err, 4x faster
than fp32 on the PE). Tokens live on the partition axis so LN/softmax
reductions are free-axis ops; x is pre-transposed on the host so the only
on-device transposes are h1/h2 (PE transpose-mode).

Fast path (biases zero, gammas one — exactly what setup_inputs produces)
exploits LN scale-invariance: relu commutes with the positive rstd, LN2
cancels LN1's rstd entirely, and LN2's rstd is deferred into the tanh
scale and the attention weights. This keeps ACT pinned to one activation
table (Copy/Relu/Tanh/Exp) except one batched Sqrt per bag.
"""

import numpy as np

B, N, D, H, C = 16, 4096, 1024, 512, 2
NCORES = 8
NB = B // NCORES       # bags per core
P = 128
NT = N // P            # token tiles per bag
DK = D // P            # k-chunks for D
HK = H // P            # k-chunks for H

_BUILD_CACHE = {}


def _build(flags):
    import concourse.bass as bass
    import concourse.mybir as mybir
    import concourse.tile as tile
    import concourse.bass_isa as bass_isa
    from concourse import bacc
    from concourse.masks import make_identity
    import contextlib

    z_b1, aff1, z_b2, aff2, z_ba1, safe_exp = flags
    fast = z_b1 and z_b2 and z_ba1 and not aff1 and not aff2 and safe_exp
    f32 = mybir.dt.float32
    f32r = mybir.dt.float32r
    EPS = 1e-5

    nc = bacc.Bacc(None, target_bir_lowering=False, debug=False)

    # f32r DRAM declarations: DMA loads straight into f32r SBUF tiles
    # (hardware rounds on PE read; verified bit-compatible with np.float32).
    xt = nc.dram_tensor("xt", [NB, D, N], f32r, kind="ExternalInput")
    W1 = nc.dram_tensor("W1", [D, H], f32r, kind="ExternalInput")
    W2 = nc.dram_tensor("W2", [H, H], f32r, kind="ExternalInput")
    Wa1 = nc.dram_tensor("Wa1", [H, H], f32r, kind="ExternalInput")
    wa2 = nc.dram_tensor("wa2", [H, 1], f32, kind="ExternalInput")
    Wc1 = nc.dram_tensor("Wc1", [H, H], f32, kind="ExternalInput")
    Wc2 = nc.dram_tensor("Wc2", [H, C], f32, kind="ExternalInput")
    bc1 = nc.dram_tensor("bc1", [H], f32, kind="ExternalInput")
    bc2 = nc.dram_tensor("bc2", [C], f32, kind="ExternalInput")
    b1 = g1 = be1 = b2 = g2 = be2 = ba1 = None
    if not z_b1:
        b1 = nc.dram_tensor("b1", [H], f32, kind="ExternalInput")
    if aff1:
        g1 = nc.dram_tensor("g1", [H], f32, kind="ExternalInput")
        be1 = nc.dram_tensor("beta1", [H], f32, kind="ExternalInput")
    if not z_b2:
        b2 = nc.dram_tensor("b2", [H], f32, kind="ExternalInput")
    if aff2:
        g2 = nc.dram_tensor("g2", [H], f32, kind="ExternalInput")
        be2 = nc.dram_tensor("beta2", [H], f32, kind="ExternalInput")
    if not z_ba1:
        ba1 = nc.dram_tensor("ba1", [H], f32, kind="ExternalInput")
    y = nc.dram_tensor("y", [NB, C], f32, kind="ExternalOutput")

    AX = mybir.AxisListType
    OP = mybir.AluOpType
    AF = mybir.ActivationFunctionType

    with tile.TileContext(nc) as tc:
        ctx = contextlib.ExitStack()
        with ctx:
            wpool = ctx.enter_context(tc.tile_pool(name="wpool", bufs=1))
            xtr = ctx.enter_context(tc.tile_pool(name="xtr", bufs=3))
            h1p = ctx.enter_context(tc.tile_pool(name="h1p", bufs=3))
            htp = ctx.enter_context(tc.tile_pool(name="htp", bufs=4))
            ap_ = ctx.enter_context(tc.tile_pool(name="ap_", bufs=3))
            h2p = ctx.enter_context(tc.tile_pool(name="h2p", bufs=NT))
            stats = ctx.enter_context(tc.tile_pool(name="stats", bufs=8))
            smallp = ctx.enter_context(tc.tile_pool(name="smallp", bufs=2))
            psmm = ctx.enter_context(tc.tile_pool(name="psmm", bufs=5, space="PSUM"))
            pstr = ctx.enter_context(tc.tile_pool(name="pstr", bufs=3, space="PSUM"))

            # ---- one-time init: identities, eps, weights in f32r ----
            ident_f = wpool.tile([P, P], f32)
            make_identity(nc, ident_f)
            ident_r = wpool.tile([P, P], f32r)
            nc.vector.tensor_copy(ident_r[:], ident_f[:])
            eps_t = wpool.tile([P, 1], f32)
            nc.vector.memset(eps_t, EPS)

            def load_conv(dram_ap, shape, name, eng=None):
                cv = wpool.tile(list(shape), f32r, name=f"wr_{name}")
                (eng or nc.sync).dma_start(cv[:], dram_ap)
                return cv

            # weight tiles are allocated up front; their DMAs are emitted
            # lazily (fast path) so the xt tile stream isn't starved at start.
            w1r = wpool.tile([P, DK, H], f32r, name="wr_w1")
            w2r = wpool.tile([P, HK, H], f32r, name="wr_w2")
            war = wpool.tile([P, HK, H], f32r, name="wr_wa1")
            wc1r = wpool.tile([P, HK, HK, P], f32, name="wr_wc1")
            wc2r = wpool.tile([P, HK, C], f32, name="wr_wc2")
            wa2_rep = wpool.tile([P, H], f32)
            bc1t = wpool.tile([P, HK], f32)
            bc2t = wpool.tile([C, 1], f32)

            def emit_w1(lo, hi):
                _w1p = W1.rearrange("(k p) h -> p k h", p=P)
                for _k in range(lo, hi):
                    nc.sync.dma_start(
                        w1r[:, _k : _k + 1, :], _w1p[:, _k : _k + 1, :]
                    )

            def emit_w2():
                nc.sync.dma_start(w2r[:], W2.rearrange("(k p) h -> p k h", p=P))

            def emit_wa1():
                nc.sync.dma_start(war[:], Wa1.rearrange("(k p) h -> p k h", p=P))

            def emit_wcls():
                nc.sync.dma_start(
                    wc1r[:], Wc1.rearrange("(k p) (m j) -> p k m j", p=P, j=P)
                )
                nc.sync.dma_start(wc2r[:], Wc2.rearrange("(k p) c -> p k c", p=P))
                nc.sync.dma_start(bc1t[:], bc1.rearrange("(m p) -> p m", p=P))
                nc.sync.dma_start(bc2t[:], bc2[:, None])
                nc.gpsimd.dma_start(
                    wa2_rep[:], wa2.rearrange("h 1 -> 1 h").to_broadcast((P, H))
                )

            def rep(v, name):
                if v is None:
                    return None
                t = wpool.tile([P, H], f32, name=f"rep_{name}")
                nc.gpsimd.dma_start(t[:], v[None, :].to_broadcast((P, H)))
                return t

            b1_rep = rep(b1, "b1")
            g1_rep = rep(g1, "g1")
            be1_rep = rep(be1, "be1")
            b2_rep = rep(b2, "b2")
            g2_rep = rep(g2, "g2")
            be2_rep = rep(be2, "be2")
            ba1_rep = rep(ba1, "ba1")

            if not fast:
                emit_w1(0, DK)
                emit_w2()
                emit_wa1()
                emit_wcls()

            xt_part = xt.rearrange("b (k p) n -> b p k n", p=P)

            # ---- general-path layernorm: full stats, relu-apply into out --
            def ln_relu(src_ps, out_sb, b_rep, g_rep, be_rep, tag):
                if b_rep is not None:
                    t = ap_.tile([P, H], f32, tag=f"lnb_{tag}", name=f"lnb_{tag}")
                    nc.vector.tensor_add(t[:], src_ps[:], b_rep[:])
                    src = t
                else:
                    src = src_ps
                bn = stats.tile([P, 6], f32, tag="bn", name="bn")
                nc.vector.bn_stats(bn[:], src[:])
                mv = stats.tile([P, 2], f32, tag="mv", name="mv")
                nc.vector.bn_aggr(mv[:], bn[:])
                sd = stats.tile([P, 1], f32, tag="sd", name="sd")
                nc.scalar.activation(sd[:], mv[:, 1:2], AF.Sqrt, bias=eps_t[:])
                rstd = stats.tile([P, 1], f32, tag="rstd", name="rstd")
                nc.vector.reciprocal(rstd[:], sd[:])
                nmr = stats.tile([P, 1], f32, tag="nmr", name="nmr")
                nc.vector.tensor_scalar(
                    nmr[:], mv[:, 0:1], rstd[:], -1.0, op0=OP.mult, op1=OP.mult
                )
                if g_rep is None:
                    nc.scalar.activation(
                        out_sb, src[:], AF.Relu, bias=nmr[:], scale=rstd[:]
                    )
                else:
                    z = ap_.tile([P, H], f32, tag=f"lnz_{tag}", name=f"lnz_{tag}")
                    nc.vector.tensor_scalar(
                        z[:], src[:], mv[:, 0:1], rstd[:], op0=OP.subtract, op1=OP.mult
                    )
                    nc.vector.tensor_mul(z[:], z[:], g_rep[:])
                    nc.vector.tensor_add(z[:], z[:], be_rep[:])
                    nc.scalar.activation(out_sb, z[:], AF.Relu)

            poolT_sb = smallp.tile([P, HK, NB], f32, bufs=1)

            # per-bag state shared between emission stages
            bag_state = [dict() for _ in range(NB)]

            XS = 4  # xt tiles per DMA super-tile (larger descriptors)

            def prefetch_super(b, start, size):
                """Issue the xt DMA for tiles [start, start+size) of bag b."""
                st = bag_state[b]
                xt_r = xtr.tile([P, DK, XS * P], f32r, tag="xtr", name="xt_r")
                nc.sync.dma_start(
                    xt_r[:, :, : size * P],
                    xt_part[b, :, :, start * P : (start + size) * P],
                )
                for t in range(size):
                    st.setdefault("xt_map", {})[start + t] = (xt_r, t * P)

            def stage_A(b, i):
                """mm1 on a prefetched xt slice, LN1 mean, relu-apply -> h1."""
                st = bag_state[b]
                xt_r, j = st["xt_map"].pop(i)
                ps1 = psmm.tile([P, H], f32, tag="mm", name="ps1")
                for k in range(DK):
                    nc.tensor.matmul(
                        ps1[:], xt_r[:, k, j : j + P], w1r[:, k, :],
                        start=(k == 0), stop=(k == DK - 1),
                    )
                h1 = h1p.tile([P, H], f32r, tag="h1", name="h1")
                # LN1 = relu(z - mean) up to a positive per-row factor that
                # LN2 cancels: H*relu(z - sum/H) = relu(H*z - sum).
                nsum1 = stats.tile([P, 1], f32, tag="sum1", name="nsum1")
                nc.vector.tensor_reduce(
                    nsum1[:], ps1[:], axis=AX.X, op=OP.add, negate=True
                )
                nc.scalar.activation(
                    h1[:], ps1[:], AF.Relu, bias=nsum1[:], scale=float(H)
                )
                st[("h1", i)] = h1

            def stage_B(b, i):
                """transpose h1, mm2, LN2 stats, relu-apply -> h2_res."""
                st = bag_state[b]
                h1 = st.pop(("h1", i))
                trp1 = pstr.tile([P, H], f32r, tag="tr", name="trp1")
                for c in range(HK):
                    nc.tensor.transpose(
                        trp1[:, c * P : (c + 1) * P],
                        h1[:, c * P : (c + 1) * P],
                        ident_r[:],
                    )
                h1T = htp.tile([P, HK, P], f32r, tag="h1T", name="h1T")
                nc.scalar.copy(h1T[:], trp1[:])
                ps2 = psmm.tile([P, H], f32, tag="mm", name="ps2")
                for k in range(HK):
                    nc.tensor.matmul(
                        ps2[:], h1T[:, k, :], w2r[:, k, :],
                        start=(k == 0), stop=(k == HK - 1),
                    )
                bn2 = stats.tile([P, 6], f32, tag="bn", name="bn2")
                nc.vector.bn_stats(bn2[:], ps2[:])
                mv2 = stats.tile([P, 2], f32, tag="mv", name="mv2")
                nc.vector.bn_aggr(mv2[:], bn2[:])
                nm2 = stats.tile([P, 1], f32, tag="nm2", name="nm2")
                nc.vector.tensor_scalar_mul(nm2[:], mv2[:, 0:1], -1.0)
                nc.gpsimd.tensor_copy(st["V"][:, i : i + 1], mv2[:, 1:2])
                # h2 holds the UNSCALED relu(z2 - m2); rstd2 is applied
                # later via the tanh scale and the attention weights.
                h2t = h2p.tile([P, H], f32r, tag="h2res", name="h2res")
                nc.scalar.activation(h2t[:], ps2[:], AF.Relu, bias=nm2[:])
                st["h2l"][i] = h2t

            def stage_rstd(b):
                """batched rstd2 = 1/sqrt(var+eps): one ACT table swap/bag."""
                st = bag_state[b]
                sd2 = smallp.tile([P, NT], f32, tag="sd2", name="sd2")
                nc.scalar.activation(sd2[:], st["V"][:], AF.Sqrt, bias=eps_t[:])
                R_sc = smallp.tile([P, NT], f32, tag="R", name="R_sc")
                nc.vector.reciprocal(R_sc[:], sd2[:])
                st["R"] = R_sc
                st["p"] = smallp.tile([P, NT], f32, tag="p", name="p_t")
                st["attn"] = smallp.tile([P, NT], f32r, tag="attn", name="attn_t")
                st["pps"] = psmm.tile([1, H], f32, tag="mm", name="pool_ps")

            def stage_C(b, i):
                """transpose h2 tile."""
                st = bag_state[b]
                trp2 = pstr.tile([P, H], f32r, tag="tr", name="trp2")
                h2t = st["h2l"][i]
                for c in range(HK):
                    nc.tensor.transpose(
                        trp2[:, c * P : (c + 1) * P],
                        h2t[:, c * P : (c + 1) * P],
                        ident_r[:],
                    )
                st[("trp2", i)] = trp2

            def stage_D(b, i):
                """mma, tanh (deferred rstd2 as scale), score dot."""
                st = bag_state[b]
                trp2 = st.pop(("trp2", i))
                h2T = htp.tile([P, HK, P], f32r, tag="h2T", name="h2T")
                if i % 2 == 0:
                    nc.scalar.copy(h2T[:], trp2[:])
                else:
                    nc.vector.tensor_copy(h2T[:], trp2[:])
                psa = psmm.tile([P, H], f32, tag="mm", name="psa")
                for k in range(HK):
                    nc.tensor.matmul(
                        psa[:], h2T[:, k, :], war[:, k, :],
                        start=(k == 0), stop=(k == HK - 1),
                    )
                a_t = ap_.tile([P, H], f32, tag="a", name="a_t")
                nc.scalar.activation(
                    a_t[:], psa[:], AF.Tanh, scale=st["R"][:, i : i + 1]
                )
                nc.gpsimd.tensor_mul(a_t[:], a_t[:], wa2_rep[:])
                nc.vector.tensor_reduce(
                    st["s"][:, i : i + 1], a_t[:], axis=AX.X, op=OP.add
                )
                # unnormalized softmax weight (no max shift: |s| bounded by
                # ||wa2||_1, checked on the host) folded with rstd2; the
                # global 1/Z lands on the pooled vector afterwards.
                nc.scalar.activation(
                    st["p"][:, i : i + 1], st["s"][:, i : i + 1], AF.Exp
                )
                nc.vector.tensor_mul(
                    st["attn"][:, i : i + 1],
                    st["p"][:, i : i + 1],
                    st["R"][:, i : i + 1],
                )
                # pooling matmul for tile i-4: slack for the
                # tanh->dot->exp chain
                if i >= 4:
                    pool_mm(b, i - 4)

            def pool_mm(b, j):
                st = bag_state[b]
                nc.tensor.matmul(
                    st["pps"][:], st["attn"][:, j : j + 1], st["h2l"][j][:],
                    start=(j == 0), stop=(j == NT - 1),
                )

            def stage_softmax(b):
                st = bag_state[b]
                zrow = stats.tile([P, 1], f32, tag="sum1", name="zrow")
                nc.vector.tensor_reduce(zrow[:], st["p"][:], axis=AX.X, op=OP.add)
                zsum = stats.tile([P, 1], f32, tag="nm1", name="zsum")
                nc.gpsimd.partition_all_reduce(
                    zsum[:], zrow[:], channels=P, reduce_op=bass_isa.ReduceOp.add
                )
                rz = stats.tile([P, 1], f32, tag="nm2", name="rz")
                nc.vector.reciprocal(rz[:], zsum[:])
                st["rz"] = rz

            def stage_pool(b):
                st = bag_state[b]
                pooled_sb = smallp.tile([P, H], f32, tag="pooled", name="pooled_sb")
                nc.vector.memset(pooled_sb[:], 0.0)
                nc.vector.tensor_copy(pooled_sb[0:1, :], st["pps"][:])
                poolT_ps = pstr.tile([P, H], f32, tag="tr", name="poolT_ps")
                for c in range(HK):
                    nc.tensor.transpose(
                        poolT_ps[:, c * P : (c + 1) * P],
                        pooled_sb[:, c * P : (c + 1) * P],
                        ident_f[:],
                    )
                # global 1/Z applied here (per-partition scalar, same value
                # on every partition)
                nc.vector.tensor_scalar_mul(
                    poolT_sb[:, :, b],
                    poolT_ps.rearrange("p (c j) -> p c j", j=P)[:, :, 0],
                    st["rz"][:],
                )

            cls_state = {}

            def cls_mm1(b):
                """per-bag half of the classifier's first GEMM (N=1)."""
                if "rc" not in cls_state:
                    cls_state["rc"] = psmm.tile(
                        [P, HK, NB], f32, tag="mm", name="rc_ps"
                    )
                rc = cls_state["rc"]
                for m in range(HK):
                    for k in range(HK):
                        nc.tensor.matmul(
                            rc[:, m, b : b + 1], wc1r[:, k, m, :],
                            poolT_sb[:, k, b : b + 1],
                            start=(k == 0), stop=(k == HK - 1),
                        )

            def cls_rest():
                rc = cls_state["rc"]
                rc_sb = smallp.tile([P, HK, NB], f32, tag="rc", name="rc_sb")
                for m in range(HK):
                    nc.scalar.activation(
                        rc_sb[:, m, :], rc[:, m, :], AF.Relu,
                        bias=bc1t[:, m : m + 1], scale=1.0,
                    )
                lg_ps = psmm.tile([C, NB], f32, tag="mm", name="lg_ps")
                for k in range(HK):
                    nc.tensor.matmul(
                        lg_ps[:], wc2r[:, k, :], rc_sb[:, k, :],
                        start=(k == 0), stop=(k == HK - 1),
                    )
                lg_sb = smallp.tile([C, NB], f32, tag="lg", name="lg_sb")
                nc.scalar.activation(
                    lg_sb[:], lg_ps[:], AF.Identity, bias=bc2t[:], scale=1.0
                )
                with nc.allow_non_contiguous_dma(reason="4-element logits store"):
                    nc.sync.dma_start(y.rearrange("b c -> c b"), lg_sb[:])

            def _scoped(fn, tag):
                def g(*a):
                    nm = tag + (str(a[1]) if len(a) > 1 else "")
                    with nc.named_scope(nm):
                        return fn(*a)
                return g

            stage_A = _scoped(stage_A, "A")
            stage_B = _scoped(stage_B, "B")
            stage_C = _scoped(stage_C, "C")
            stage_D = _scoped(stage_D, "D")
            stage_rstd = _scoped(stage_rstd, "rstd")
            stage_softmax = _scoped(stage_softmax, "sm")
            stage_pool = _scoped(stage_pool, "pool")
            prefetch_super = _scoped(prefetch_super, "pf")

            if fast:
                # Software-pipelined emission: Tile dispatches each engine in
                # emission order, so skew the stages to keep PE fed.
                PRE = 2  # tiles of the next bag's phase 1 emitted before pooling
                for b in range(NB):
                    st = bag_state[b]
                    if b == 0:
                        st["h2l"] = {}
                        st["s"] = smallp.tile([P, NT], f32, tag="s", name="s_sc")
                        st["V"] = smallp.tile([P, NT], f32, tag="V", name="V_sc")
                        # small first super so the very first matmul starts
                        # early; weight loads staged between xt supers
                        prefetch_super(0, 0, 1)
                        emit_w1(0, 3)
                        prefetch_super(0, 1, 3)
                        emit_w1(3, DK)
                    for i in (range(NT) if b == 0 else range(PRE + 4, NT)):
                        ns = i + XS
                        if ns < NT and ns % XS == 0:
                            prefetch_super(b, ns, XS)
                        if i == NT - XS and b + 1 < NB:
                            prefetch_super(b + 1, 0, XS)
                        stage_A(b, i)
                        if b == 0 and i == 1:
                            emit_w2()
                        if b == 0 and i == 4:
                            emit_wa1()
                        if b == 0 and i == 8:
                            emit_wcls()
                        if i >= 2:
                            stage_B(b, i - 2)
                    stage_B(b, NT - 2)
                    stage_B(b, NT - 1)
                    # pre-emit the next bag's first tiles here: their mm1s
                    # cover the PE while ACT drains the LN2/rstd chain.
                    if b + 1 < NB:
                        nb_ = b + 1
                        stn = bag_state[nb_]
                        stn["h2l"] = {}
                        stn["s"] = smallp.tile([P, NT], f32, tag="s", name="s_sc")
                        stn["V"] = smallp.tile([P, NT], f32, tag="V", name="V_sc")
                        prefetch_super(nb_, XS, XS)
                        for i in range(PRE):
                            stage_A(nb_, i)
                    stage_rstd(b)
                    for i in range(NT):
                        stage_C(b, i)
                        if i >= 2:
                            stage_D(b, i - 2)
                    stage_D(b, NT - 2)
                    stage_D(b, NT - 1)
                    for j in range(NT - 4, NT):
                        pool_mm(b, j)
                    stage_softmax(b)
                    # cover the softmax->pool latency with next-bag phase-1
                    # tiles (or, for the last bag, the classifier's first GEMM)
                    if b + 1 < NB:
                        for i in range(PRE, PRE + 4):
                            ns = i + XS
                            if ns < NT and ns % XS == 0:
                                prefetch_super(b + 1, ns, XS)
                            stage_A(b + 1, i)
                            stage_B(b + 1, i - PRE)
                    else:
                        cls_mm1(0)
                    stage_pool(b)
            else:
                for b in range(NB):
                    h2_res = h2p.tile(
                        [P, NT, H], f32r, tag="h2big", name="h2res", bufs=1
                    )
                    s_sc = smallp.tile([P, NT], f32, tag="s", name="s_sc")

                    for i in range(NT):
                        xt_r = xtr.tile([P, DK, P], f32r, tag="xtr", name="xt_r")
                        nc.sync.dma_start(
                            xt_r[:], xt_part[b, :, :, i * P : (i + 1) * P]
                        )
                        ps1 = psmm.tile([P, H], f32, tag="mm", name="ps1")
                        for k in range(DK):
                            nc.tensor.matmul(
                                ps1[:], xt_r[:, k, :], w1r[:, k, :],
                                start=(k == 0), stop=(k == DK - 1),
                            )
                        h1 = h1p.tile([P, H], f32r, tag="h1", name="h1")
                        ln_relu(ps1, h1[:], b1_rep, g1_rep, be1_rep, "1")

                        trp1 = pstr.tile([P, H], f32r, tag="tr", name="trp1")
                        for c in range(HK):
                            nc.tensor.transpose(
                                trp1[:, c * P : (c + 1) * P],
                                h1[:, c * P : (c + 1) * P],
                                ident_r[:],
                            )
                        h1T = htp.tile([P, HK, P], f32r, tag="h1T", name="h1T")
                        nc.scalar.copy(h1T[:], trp1[:])

                        ps2 = psmm.tile([P, H], f32, tag="mm", name="ps2")
                        for k in range(HK):
                            nc.tensor.matmul(
                                ps2[:], h1T[:, k, :], w2r[:, k, :],
                                start=(k == 0), stop=(k == HK - 1),
                            )
                        ln_relu(ps2, h2_res[:, i, :], b2_rep, g2_rep, be2_rep, "2")

                        trp2 = pstr.tile([P, H], f32r, tag="tr", name="trp2")
                        for c in range(HK):
                            nc.tensor.transpose(
                                trp2[:, c * P : (c + 1) * P],
                                h2_res[:, i, c * P : (c + 1) * P],
                                ident_r[:],
                            )
                        h2T = htp.tile([P, HK, P], f32r, tag="h2T", name="h2T")
                        nc.vector.tensor_copy(h2T[:], trp2[:])

                        psa = psmm.tile([P, H], f32, tag="mm", name="psa")
                        for k in range(HK):
                            nc.tensor.matmul(
                                psa[:], h2T[:, k, :], war[:, k, :],
                                start=(k == 0), stop=(k == HK - 1),
                            )
                        a_t = ap_.tile([P, H], f32, tag="a", name="a_t")
                        if ba1_rep is not None:
                            nc.vector.tensor_add(a_t[:], psa[:], ba1_rep[:])
                            nc.scalar.activation(a_t[:], a_t[:], AF.Tanh)
                        else:
                            nc.scalar.activation(a_t[:], psa[:], AF.Tanh)
                        nc.gpsimd.tensor_mul(a_t[:], a_t[:], wa2_rep[:])
                        nc.vector.tensor_reduce(
                            s_sc[:, i : i + 1], a_t[:], axis=AX.X, op=OP.add
                        )

                    rmax = stats.tile([P, 1], f32, tag="sum1", name="rmax")
                    nc.vector.tensor_reduce(rmax[:], s_sc[:], axis=AX.X, op=OP.max)
                    gmax = stats.tile([P, 1], f32, tag="nm1", name="gmax")
                    nc.gpsimd.partition_all_reduce(
                        gmax[:], rmax[:], channels=P, reduce_op=bass_isa.ReduceOp.max
                    )
                    ngmax = stats.tile([P, 1], f32, tag="nm2", name="ngmax")
                    nc.vector.tensor_scalar_mul(ngmax[:], gmax[:], -1.0)
                    p_t = smallp.tile([P, NT], f32, tag="p", name="p_t")
                    zrow = stats.tile([P, 1], f32, tag="sum1", name="zrow")
                    nc.scalar.activation(
                        p_t[:], s_sc[:], AF.Exp, bias=ngmax[:], scale=1.0,
                        accum_out=zrow[:],
                    )
                    zsum = stats.tile([P, 1], f32, tag="nm1", name="zsum")
                    nc.gpsimd.partition_all_reduce(
                        zsum[:], zrow[:], channels=P, reduce_op=bass_isa.ReduceOp.add
                    )
                    rz = stats.tile([P, 1], f32, tag="nm2", name="rz")
                    nc.vector.reciprocal(rz[:], zsum[:])
                    attn_t = smallp.tile([P, NT], f32r, tag="attn", name="attn_t")
                    nc.vector.tensor_scalar_mul(attn_t[:], p_t[:], rz[:])

                    pool_ps = psmm.tile([1, H], f32, tag="mm", name="pool_ps")
                    for i in range(NT):
                        nc.tensor.matmul(
                            pool_ps[:], attn_t[:, i : i + 1], h2_res[:, i, :],
                            start=(i == 0), stop=(i == NT - 1),
                        )
                    pooled_sb = smallp.tile([P, H], f32, tag="pooled", name="pooled_sb")
                    nc.vector.memset(pooled_sb[:], 0.0)
                    nc.vector.tensor_copy(pooled_sb[0:1, :], pool_ps[:])
                    poolT_ps = pstr.tile([P, H], f32, tag="tr", name="poolT_ps")
                    for c in range(HK):
                        nc.tensor.transpose(
                            poolT_ps[:, c * P : (c + 1) * P],
                            pooled_sb[:, c * P : (c + 1) * P],
                            ident_f[:],
                        )
                    nc.vector.tensor_copy(
                        poolT_sb[:, :, b],
                        poolT_ps.rearrange("p (c j) -> p c j", j=P)[:, :, 0],
                    )

            # ---- classifier tail ----
            if fast:
                cls_mm1(NB - 1)
                cls_rest()
            else:
                cls_mm1(0)
                cls_mm1(NB - 1)
                cls_rest()

    nc.compile()
    return nc


def _get_program(flags):
    if flags not in _BUILD_CACHE:
        _BUILD_CACHE[flags] = _build(flags)
    return _BUILD_CACHE[flags]


def kernel(**inputs):
    import sys
    for pth in ("/opt/trn_rl_repo",):
        if pth not in sys.path:
            sys.path.append(pth)
    from concourse.bass_utils import run_bass_kernel_spmd

    x = np.asarray(inputs["x"], dtype=np.float32)
    names = ["W1", "b1", "g1", "beta1", "W2", "b2", "g2", "beta2",
             "Wa1", "ba1", "wa2", "ba2", "Wc1", "bc1", "Wc2", "bc2"]
    w = {k: np.asarray(inputs[k], dtype=np.float32) for k in names}

    z_b1 = bool((w["b1"] == 0).all())
    aff1 = not bool((w["g1"] == 1).all() and (w["beta1"] == 0).all())
    z_b2 = bool((w["b2"] == 0).all())
    aff2 = not bool((w["g2"] == 1).all() and (w["beta2"] == 0).all())
    z_ba1 = bool((w["ba1"] == 0).all())
    # no-max-shift softmax is safe iff scores can't overflow exp in fp32
    safe_exp = bool(np.abs(w["wa2"]).sum() < 60.0)
    flags = (z_b1, aff1, z_b2, aff2, z_ba1, safe_exp)

    nc = _get_program(flags)

    in_maps = []
    for core in range(NCORES):
        shard = x[core * NB : (core + 1) * NB]          # [NB, N, D]
        xtr = np.ascontiguousarray(shard.transpose(0, 2, 1))  # [NB, D, N]
        m = {
            "xt": xtr,
            "W1": w["W1"], "W2": w["W2"], "Wa1": w["Wa1"],
            "wa2": w["wa2"].reshape(H, 1),
            "Wc1": w["Wc1"], "Wc2": w["Wc2"],
            "bc1": w["bc1"], "bc2": w["bc2"],
        }
        if not z_b1:
            m["b1"] = w["b1"]
        if aff1:
            m["g1"] = w["g1"]
            m["beta1"] = w["beta1"]
        if not z_b2:
            m["b2"] = w["b2"]
        if aff2:
            m["g2"] = w["g2"]
            m["beta2"] = w["beta2"]
        if not z_ba1:
            m["ba1"] = w["ba1"]
        in_maps.append(m)

    res = run_bass_kernel_spmd(nc, in_maps, core_ids=list(range(NCORES)))
    out = np.concatenate([res.results[i]["y"] for i in range(NCORES)], axis=0)
    return out.astype(np.float32)
